# revision 10
# baseline (speedup 1.0000x reference)
# Involution2d (K=7) Trainium2 kernel — 8-core SPMD, batch+spatial sharding.
#
# Sharding: 8 cores = (batch b in 0..3) x (H-half in 0..1); each core owns a
# [128, 32, 64] output block (2048 pixels, p = 64*h + w).
#
# Per-core algorithm (banded pixel->pixel matrix, v2):
#   1. gen (bf16): 1x1 conv (BN folded) -> ReLU -> 1x1 conv, emitted directly
#      in pixel-major layout kermT[p, o] (16 matmuls of [33,128]^T @ [33,49];
#      bias rides an ones-row in the stationary operand).
#   2. GPSIMD local_scatter per 128-pixel tile mb: place the 49 kernel values
#      of pixel p at column q - 128*mb of A2T[p, :], where q = p + 192 +
#      64*di + dj is the flattened source pixel (38 rows x 64 cols q-space;
#      halo rows from the neighbor core, zeros at image edges). W-edge terms
#      get idx=-1 (skipped), which provably clips the window to 512 columns.
#   3. q-major strips st[mb][q, 128j + p] = A2[q-tile mb+j][q, p] built per
#      tile either by ONE XBAR DMA transpose (dst rearranged "q (j p)"),
#      or by 4 TensorE 128x128 transposes + PSUM->SBUF copies (hybrid split
#      keeps PE, queue engines, and DVE/Scalar all below the window).
#   4. involution mb-major: po[g][c, 128(mb%4)+p] += xtp_tile(mb+j)^T @
#      st[mb][:, 128j:128j+128] for j=0..3 (accumulating PSUM bank per 4
#      tiles), then cast to bf16 (DVE+Scalar halves) and DMA out per group.
import numpy as np
import ml_dtypes

EPS = 1e-5
KK = 7
C = 128
H = 64
W = 64
B = 4
HH = 32            # rows per core
P = HH * W         # 2048 output pixels per core
NQT = 19           # q tiles: (HH + 6) * W / 128
NO = 50            # offset count padded to even (49 + 1 dummy)
AWIN = 512         # scatter window (4 q-tiles)

# packed-constants byte layout (per partition)
OFF_W1 = 0         # [128, 32] bf16      -> 64 B
OFF_ID = 64        # [128, 128] bf16     -> 256 B (transpose identity)
OFF_I2 = 320       # [128, 200] int16    -> 400 B (4-tile scatter table)
OFF_B1 = 720       # [32, 1] f32         -> 4 B
OFF_W2 = 724       # [33, 49] bf16       -> 98 B
OFF_X0 = 824       # [128, 128] bf16     -> 256 B (x pixel-tile 0, c-major)
NCB = 1080

# tiles whose strips are built by XBAR DMA transpose (rest: TensorE)
XBAR_TILES = frozenset()

# gen chunks (in 128-px tiles): a small first chunk starts the GPSIMD
# scatter chain as early as possible
CHUNK_TILES = ((0,), (1, 2, 3), (4, 5, 6, 7), (8, 9, 10, 11),
               (12, 13, 14, 15))
SCATTER_BATCHES = ((0,), (1, 2, 3), (4, 5, 6), (7, 8, 9), (10, 11, 12),
                   (13, 14), (15,))

_STATE = {}

BF16 = ml_dtypes.bfloat16


def _build():
    import concourse.tile as tile
    from concourse import bacc, mybir

    f32 = mybir.dt.float32
    bf16 = mybir.dt.bfloat16
    i16 = mybir.dt.int16
    u8 = mybir.dt.uint8
    u32 = mybir.dt.uint32
    nc = bacc.Bacc("TRN2", target_bir_lowering=False, debug=False)

    XCHUNKS = tuple(128 * len(ts) for ts in CHUNK_TILES)
    xcm_d = [
        nc.dram_tensor(f"xcm{i}", [C, n], bf16, kind="ExternalInput").ap()
        for i, n in enumerate(XCHUNKS) if i > 0
    ]
    xcm_d.insert(0, None)
    xtp_d = [
        nc.dram_tensor(f"xtp{i}", [128, n * 128], bf16, kind="ExternalInput").ap()
        for i, n in ((0, 10), (1, 9))
    ]
    cb_d = nc.dram_tensor("cb", [128, NCB], u8, kind="ExternalInput").ap()
    out_d = nc.dram_tensor("out", [C, P], bf16, kind="ExternalOutput").ap()

    with tile.TileContext(nc) as tc:
        with (
            tc.tile_pool(name="consts", bufs=1) as cpool,
            tc.tile_pool(name="pgen", bufs=2, space="PSUM") as pgen,
            tc.tile_pool(name="pkt", bufs=2, space="PSUM") as pkt,
            tc.tile_pool(name="ptp", bufs=2, space="PSUM") as ptp,
            tc.tile_pool(name="pout", bufs=2, space="PSUM") as pout,
        ):
            # --- input DMAs interleaved across both queues so the gen path
            # (cb + xcm chunks, in chunk order) lands first; the big xtp
            # tensors stream behind them ---
            cb = cpool.tile([128, NCB], u8, tag="cb")
            xcm = [None] + [
                cpool.tile([C, n], bf16, tag=f"xcm{i}", name=f"xcm{i}")
                for i, n in enumerate(XCHUNKS) if i > 0]
            xtp = [cpool.tile([128, n * 128], bf16, tag=f"xtp{i}", name=f"xtp{i}")
                   for i, n in ((0, 10), (1, 9))]
            nc.sync.dma_start(cb[:], cb_d)
            nc.scalar.dma_start(xcm[1][:], xcm_d[1])
            nc.sync.dma_start(xcm[2][:], xcm_d[2])
            nc.scalar.dma_start(xcm[3][:], xcm_d[3])
            nc.sync.dma_start(xcm[4][:], xcm_d[4])
            nc.scalar.dma_start(xtp[0][:], xtp_d[0])
            nc.sync.dma_start(xtp[1][:], xtp_d[1])

            # HAM warm-up: the PE clock-gate runs at 1.2 GHz until it sees a
            # full ~3.4us busy window; without priming, the kernel's bursty
            # gen phase keeps the PE cold until late (K=4/8 until ~23us in
            # the baseline trace). Dummy matmuls on a zeroed tile fill the
            # DMA-wait prologue and the gen-phase gaps so the real transpose
            # + involution stream runs at 2.4 GHz.
            dum = cpool.tile([128, 128], bf16, tag="dum")
            nc.vector.memset(dum[:].bitcast(u32), 0)
            pdum = pgen.tile([32, 512], f32, tag="f1", name="pdum")

            def warm(n, dep=None):
                # dep pins the dummy's position in the in-order TensorE
                # queue: without a data dependency the Tile scheduler hoists
                # dummies ahead of real work and delays the gen chain.
                for _ in range(n):
                    if dep is None:
                        lhsT, rhs = dum[:, 0:32], dum[:]
                    else:
                        pp = dep.partition_size()
                        w = min(dep.shape[1], 128)
                        lhsT = dep[0:pp, 0:32]
                        rhs = dep[0:pp, 0:w]
                    nc.tensor.matmul(pdum[0:32, 0:rhs.shape[1]], lhsT, rhs,
                                     start=True, stop=True,
                                     skip_group_check=True)

            warm(24)

            # Prefetch the GPSIMD local_scatter ucode library (see baseline).
            libw = cpool.tile([128, 2], bf16, tag="libw")
            libi = cpool.tile([128, 2], i16, tag="libi")
            nc.vector.memset(libi[:].bitcast(u32), 0xFFFFFFFF)
            nc.gpsimd.local_scatter(libw[:], libw[:], libi[:],
                                    channels=128, num_elems=2, num_idxs=2)

            w1sT = cb[:, OFF_W1:OFF_W1 + 64].bitcast(bf16)       # [128, 32]
            ident = cb[:, OFF_ID:OFF_ID + 256].bitcast(bf16)     # [128, 128]
            idxt4 = cb[:, OFF_I2:OFF_I2 + 400].bitcast(i16)      # [128, 200]
            b1f = cb[0:32, OFF_B1:OFF_B1 + 4].bitcast(f32)       # [32, 1]
            w2b = cb[0:33, OFF_W2:OFF_W2 + 98].bitcast(bf16)     # [33, 49]
            xcm0 = cb[:, OFF_X0:OFF_X0 + 256].bitcast(bf16)      # [128, 128]

            def xtp_tile(kb):
                return (xtp[0][:, kb * 128:(kb + 1) * 128] if kb < 10
                        else xtp[1][:, (kb - 10) * 128:(kb - 9) * 128])

            outsb = cpool.tile([C, P], bf16, tag="outsb")
            fb = cpool.tile([33, P], bf16, tag="fb")
            nc.vector.memset(fb[32:33, :].bitcast(u32), 0x3F803F80)
            kermT = cpool.tile([128, 16 * NO], bf16, tag="kermT")

            a2v = [None] * 16

            po = {}

            def emit_strip_and_mms(mb):
                st = cpool.tile([128, AWIN], bf16, name=f"st{mb % 4}",
                                tag=f"st{mb % 4}")
                if mb in XBAR_TILES:
                    dst = st[:].rearrange("q (j p) -> q j p", p=128)
                    eng = nc.sync if mb % 2 == 0 else nc.scalar
                    eng.dma_start(dst, a2v[mb], transpose=True)
                else:
                    tp = ptp.tile([128, AWIN], bf16, tag="tp")
                    for j in range(4):
                        nc.tensor.transpose(
                            tp[:, j * 128:(j + 1) * 128],
                            a2v[mb][:, j * 128:(j + 1) * 128],
                            ident,
                        )
                    nc.vector.tensor_copy(st[:, 0:256], tp[:, 0:256])
                    nc.scalar.copy(st[:, 256:512], tp[:, 256:512])
                g = mb // 4
                first = g not in po
                if first:
                    po[g] = pout.tile([C, 512], f32, name=f"po{g}", tag="po")
                col = 128 * (mb % 4)
                for j in range(4):
                    nc.tensor.matmul(
                        po[g][:, col:col + 128],
                        xtp_tile(mb + j),
                        st[:, 128 * j:128 * (j + 1)],
                        start=(first and j == 0),
                        stop=(mb % 4 == 3 and j == 3),
                        skip_group_check=True,
                    )
                if mb % 4 == 3:
                    glo = 512 * g
                    nc.vector.tensor_copy(
                        outsb[:, glo:glo + 256], po[g][:, 0:256])
                    nc.scalar.copy(
                        outsb[:, glo + 256:glo + 512], po[g][:, 256:512])
                    eng = nc.sync if g % 2 == 0 else nc.scalar
                    eng.dma_start(out_d[:, glo:glo + 512],
                                  outsb[:, glo:glo + 512])

            cast_done = 0
            batch_it = iter(SCATTER_BATCHES)
            pending = next(batch_it)
            off = 0
            for ci, n in enumerate(XCHUNKS):
                fsl = slice(off, off + n)
                off += n
                f1 = pgen.tile([32, 512], f32, tag="f1")
                xsrc = xcm0 if ci == 0 else xcm[ci][:]
                nc.tensor.matmul(f1[:, 0:n], w1sT, xsrc,
                                 start=True, stop=True)
                nc.scalar.activation(
                    fb[0:32, fsl], f1[:, 0:n],
                    mybir.ActivationFunctionType.Relu, bias=b1f,
                )
                for t in CHUNK_TILES[ci]:
                    kt = pkt.tile([128, 512], f32, tag="kt")
                    nc.tensor.matmul(
                        kt[:, 0:49], fb[:, 128 * t:128 * (t + 1)], w2b,
                        start=True, stop=True,
                    )
                    nc.vector.tensor_copy(kermT[:, t * NO:t * NO + 49],
                                          kt[:, 0:49])
                    warm(1, dep=kermT[:, t * NO:t * NO + 49])
                cast_done = CHUNK_TILES[ci][-1] + 1
                while pending is not None and pending[-1] < cast_done:
                    mbs_b = pending
                    k0 = mbs_b[0]
                    nb = len(mbs_b)
                    ab = cpool.tile([128, nb * AWIN], bf16, name=f"a2b{k0}",
                                    tag=f"a2b{k0}")
                    nc.gpsimd.local_scatter(
                        ab[:], kermT[:, k0 * NO:(k0 + nb) * NO],
                        idxt4[:, 0:nb * NO],
                        channels=128, num_elems=nb * AWIN, num_idxs=nb * NO,
                    )
                    for j, mb in enumerate(mbs_b):
                        a2v[mb] = ab[:, j * AWIN:(j + 1) * AWIN]
                    for mb in mbs_b:
                        emit_strip_and_mms(mb)
                        warm(1, dep=kermT[:, 0:128])
                    pending = next(batch_it, None)

    nc.compile()
    return nc


def _get_nc():
    if "nc" not in _STATE:
        _STATE["nc"] = _build()
    return _STATE["nc"]


def _make_idx_table():
    p_loc = np.arange(128)[:, None]
    o = np.arange(49)[None, :]
    di = o // 7 - 3
    dj = o % 7 - 3
    w_of = p_loc % 64
    idx = p_loc + 192 + 64 * di + dj
    masked = (w_of + dj < 0) | (w_of + dj >= 64)
    idx = np.where(masked, -1, idx)
    tab = np.full((128, NO), -1, dtype=np.int16)
    tab[:, :49] = idx.astype(np.int16)
    return tab


def _host_prep(x, w1, b1, bn_gamma, bn_beta, bn_mean, bn_var, w2, b2):
    x = np.asarray(x, dtype=np.float32)
    scale = np.asarray(bn_gamma) / np.sqrt(np.asarray(bn_var) + EPS)
    w1s = (np.asarray(w1) * scale[:, None]).astype(np.float32)
    b1f = (np.asarray(b1) * scale + np.asarray(bn_beta)
           - np.asarray(bn_mean) * scale).astype(np.float32)
    w1sT = np.ascontiguousarray(w1s.T).astype(BF16)            # [128, 32]
    w2b = np.vstack([np.asarray(w2, np.float32).T,
                     np.asarray(b2, np.float32)[None, :]]).astype(BF16)  # [33, 49]
    idxt = _make_idx_table()                                   # [128, 50] i16
    ident = np.eye(128, dtype=np.float32).astype(BF16)

    cb = np.zeros((128, NCB), np.uint8)
    cb[:, OFF_W1:OFF_W1 + 64] = np.ascontiguousarray(w1sT).view(np.uint8)
    cb[:, OFF_ID:OFF_ID + 256] = np.ascontiguousarray(ident).view(np.uint8)
    idxt4 = np.concatenate(
        [np.where(idxt >= 0, idxt + AWIN * j, -1) for j in range(4)],
        axis=1).astype(np.int16)
    cb[:, OFF_I2:OFF_I2 + 400] = idxt4.view(np.uint8)
    cb[0:32, OFF_B1:OFF_B1 + 4] = np.ascontiguousarray(
        b1f[:, None]).view(np.uint8)
    cb[0:33, OFF_W2:OFF_W2 + 98] = np.ascontiguousarray(w2b).view(np.uint8)

    in_maps = []
    for core in range(8):
        b, half = divmod(core, 2)
        h0 = HH * half
        xcm = np.ascontiguousarray(
            x[b, :, h0:h0 + HH, :].reshape(C, P)).astype(BF16)
        cbc = cb.copy()
        cbc[:, OFF_X0:OFF_X0 + 256] = np.ascontiguousarray(
            xcm[:, 0:128]).view(np.uint8)
        # q-space: rows h0-3 .. h0+35 (zeros outside the image)
        xe = np.zeros((C, HH + 6, W), dtype=np.float32)
        lo = max(0, h0 - 3)
        hi = min(H, h0 + HH + 3)
        xe[:, lo - (h0 - 3):hi - (h0 - 3), :] = x[b, :, lo:hi, :]
        xq = xe.reshape(C, NQT * 128).T                        # [2432, 128]
        xtp = np.ascontiguousarray(
            xq.reshape(NQT, 128, 128).transpose(1, 0, 2).reshape(128, NQT * 128)
        ).astype(BF16)
        bounds = np.cumsum((0,) + tuple(128 * len(ts) for ts in CHUNK_TILES))
        im = {f"xcm{i}": xcm[:, bounds[i]:bounds[i + 1]]
              for i in range(1, len(CHUNK_TILES))}
        im.update({"xtp0": xtp[:, :10 * 128], "xtp1": xtp[:, 10 * 128:],
                   "cb": cbc})
        in_maps.append(im)
    return in_maps


def run(inputs: dict, trace: bool = False):
    from concourse.bass_utils import run_bass_kernel_spmd

    nc = _get_nc()
    in_maps = _host_prep(**inputs)
    res = run_bass_kernel_spmd(
        nc, in_maps, core_ids=list(range(8)), trace=trace,
    )
    out = np.zeros((B, C, H, W), dtype=np.float32)
    for core in range(8):
        b, half = divmod(core, 2)
        h0 = HH * half
        out[b, :, h0:h0 + HH, :] = (
            res.results[core]["out"].astype(np.float32).reshape(C, HH, W)
        )
    return out, res


def kernel(**inputs) -> np.ndarray:
    out, _ = run(inputs, trace=False)
    return out


# revision 11
# speedup vs baseline: 1.0408x; 1.0408x over previous
# Involution2d (K=7) Trainium2 kernel — 8-core SPMD, batch+spatial sharding.
#
# Sharding: 8 cores = (batch b in 0..3) x (H-half in 0..1); each core owns a
# [128, 32, 64] output block (2048 pixels, p = 64*h + w).
#
# Per-core algorithm (banded pixel->pixel matrix, v2):
#   1. gen (bf16): 1x1 conv (BN folded) -> ReLU -> 1x1 conv, emitted directly
#      in pixel-major layout kermT[p, o] (16 matmuls of [33,128]^T @ [33,49];
#      bias rides an ones-row in the stationary operand).
#   2. GPSIMD local_scatter per 128-pixel tile mb: place the 49 kernel values
#      of pixel p at column q - 128*mb of A2T[p, :], where q = p + 192 +
#      64*di + dj is the flattened source pixel (38 rows x 64 cols q-space;
#      halo rows from the neighbor core, zeros at image edges). W-edge terms
#      get idx=-1 (skipped), which provably clips the window to 512 columns.
#   3. q-major strips st[mb][q, 128j + p] = A2[q-tile mb+j][q, p] built per
#      tile either by ONE XBAR DMA transpose (dst rearranged "q (j p)"),
#      or by 4 TensorE 128x128 transposes + PSUM->SBUF copies (hybrid split
#      keeps PE, queue engines, and DVE/Scalar all below the window).
#   4. involution mb-major: po[g][c, 128(mb%4)+p] += xtp_tile(mb+j)^T @
#      st[mb][:, 128j:128j+128] for j=0..3 (accumulating PSUM bank per 4
#      tiles), then cast to bf16 (DVE+Scalar halves) and DMA out per group.
import numpy as np
import ml_dtypes

EPS = 1e-5
KK = 7
C = 128
H = 64
W = 64
B = 4
HH = 32            # rows per core
P = HH * W         # 2048 output pixels per core
NQT = 19           # q tiles: (HH + 6) * W / 128
NO = 50            # offset count padded to even (49 + 1 dummy)
AWIN = 512         # scatter window (4 q-tiles)

# packed-constants byte layout (per partition)
OFF_W1 = 0         # [128, 32] bf16      -> 64 B
OFF_ID = 64        # [128, 128] bf16     -> 256 B (transpose identity)
OFF_I2 = 320       # [128, 200] int16    -> 400 B (4-tile scatter table)
OFF_B1 = 720       # [32, 1] f32         -> 4 B
OFF_W2 = 724       # [33, 49] bf16       -> 98 B
OFF_X0 = 824       # [128, 128] bf16     -> 256 B (x pixel-tile 0, c-major)
NCB = 1080

# tiles whose strips are built by XBAR DMA transpose (rest: TensorE)
XBAR_TILES = frozenset()

# gen chunks (in 128-px tiles): a small first chunk starts the GPSIMD
# scatter chain as early as possible
CHUNK_TILES = ((0,), (1, 2, 3), (4, 5, 6, 7), (8, 9, 10, 11),
               (12, 13, 14, 15))
SCATTER_BATCHES = ((0,), (1, 2, 3), (4, 5, 6), (7, 8, 9), (10, 11, 12),
                   (13, 14), (15,))

_STATE = {}

BF16 = ml_dtypes.bfloat16


def _build():
    import concourse.tile as tile
    from concourse import bacc, mybir

    f32 = mybir.dt.float32
    bf16 = mybir.dt.bfloat16
    i16 = mybir.dt.int16
    u8 = mybir.dt.uint8
    u32 = mybir.dt.uint32
    nc = bacc.Bacc("TRN2", target_bir_lowering=False, debug=False)

    XCHUNKS = tuple(128 * len(ts) for ts in CHUNK_TILES)
    xcm_d = [
        nc.dram_tensor(f"xcm{i}", [C, n], bf16, kind="ExternalInput").ap()
        for i, n in enumerate(XCHUNKS) if i > 0
    ]
    xcm_d.insert(0, None)
    xtp_d = [
        nc.dram_tensor(f"xtp{i}", [128, n * 128], bf16, kind="ExternalInput").ap()
        for i, n in ((0, 10), (1, 9))
    ]
    cb_d = nc.dram_tensor("cb", [128, NCB], u8, kind="ExternalInput").ap()
    out_d = nc.dram_tensor("out", [C, P], bf16, kind="ExternalOutput").ap()

    with tile.TileContext(nc) as tc:
        with (
            tc.tile_pool(name="consts", bufs=1) as cpool,
            tc.tile_pool(name="pgen", bufs=2, space="PSUM") as pgen,
            tc.tile_pool(name="pkt", bufs=2, space="PSUM") as pkt,
            tc.tile_pool(name="ptp", bufs=2, space="PSUM") as ptp,
            tc.tile_pool(name="pout", bufs=2, space="PSUM") as pout,
        ):
            # --- input DMAs interleaved across both queues so the gen path
            # (cb + xcm chunks, in chunk order) lands first; the big xtp
            # tensors stream behind them ---
            cb = cpool.tile([128, NCB], u8, tag="cb")
            xcm = [None] + [
                cpool.tile([C, n], bf16, tag=f"xcm{i}", name=f"xcm{i}")
                for i, n in enumerate(XCHUNKS) if i > 0]
            xtp = [cpool.tile([128, n * 128], bf16, tag=f"xtp{i}", name=f"xtp{i}")
                   for i, n in ((0, 10), (1, 9))]
            nc.sync.dma_start(cb[:], cb_d)
            nc.scalar.dma_start(xcm[1][:], xcm_d[1])
            nc.sync.dma_start(xcm[2][:], xcm_d[2])
            nc.scalar.dma_start(xcm[3][:], xcm_d[3])
            nc.sync.dma_start(xcm[4][:], xcm_d[4])
            nc.scalar.dma_start(xtp[0][:], xtp_d[0])
            nc.sync.dma_start(xtp[1][:], xtp_d[1])

            # HAM warm-up: the PE clock-gate runs at 1.2 GHz until it sees a
            # full ~3.4us busy window; without priming, the kernel's bursty
            # gen phase keeps the PE cold until late (K=4/8 until ~23us in
            # the baseline trace). Dummy matmuls on a zeroed tile fill the
            # DMA-wait prologue and the gen-phase gaps so the real transpose
            # + involution stream runs at 2.4 GHz.
            dum = cpool.tile([128, 128], bf16, tag="dum")
            nc.vector.memset(dum[:].bitcast(u32), 0)
            pdum = pgen.tile([32, 512], f32, tag="f1", name="pdum")

            def warm(n, dep=None):
                # dep pins the dummy's position in the in-order TensorE
                # queue: without a data dependency the Tile scheduler hoists
                # dummies ahead of real work and delays the gen chain.
                for _ in range(n):
                    if dep is None:
                        lhsT, rhs = dum[:, 0:32], dum[:]
                    else:
                        pp = dep.partition_size()
                        w = min(dep.shape[1], 128)
                        lhsT = dep[0:pp, 0:32]
                        rhs = dep[0:pp, 0:w]
                    nc.tensor.matmul(pdum[0:32, 0:rhs.shape[1]], lhsT, rhs,
                                     start=True, stop=True,
                                     skip_group_check=True)

            warm(24)

            # Prefetch the GPSIMD local_scatter ucode library (see baseline).
            libw = cpool.tile([128, 2], bf16, tag="libw")
            libi = cpool.tile([128, 2], i16, tag="libi")
            nc.vector.memset(libi[:].bitcast(u32), 0xFFFFFFFF)
            nc.gpsimd.local_scatter(libw[:], libw[:], libi[:],
                                    channels=128, num_elems=2, num_idxs=2)

            w1sT = cb[:, OFF_W1:OFF_W1 + 64].bitcast(bf16)       # [128, 32]
            ident = cb[:, OFF_ID:OFF_ID + 256].bitcast(bf16)     # [128, 128]
            idxt4 = cb[:, OFF_I2:OFF_I2 + 400].bitcast(i16)      # [128, 200]
            b1f = cb[0:32, OFF_B1:OFF_B1 + 4].bitcast(f32)       # [32, 1]
            w2b = cb[0:33, OFF_W2:OFF_W2 + 98].bitcast(bf16)     # [33, 49]
            xcm0 = cb[:, OFF_X0:OFF_X0 + 256].bitcast(bf16)      # [128, 128]

            def xtp_tile(kb):
                return (xtp[0][:, kb * 128:(kb + 1) * 128] if kb < 10
                        else xtp[1][:, (kb - 10) * 128:(kb - 9) * 128])

            outsb = cpool.tile([C, P], bf16, tag="outsb")
            fb = cpool.tile([33, P], bf16, tag="fb")
            nc.vector.memset(fb[32:33, :].bitcast(u32), 0x3F803F80)
            kermT = cpool.tile([128, 16 * NO], bf16, tag="kermT")

            a2v = [None] * 16

            po = {}

            def emit_strip_and_mms(mb):
                st = cpool.tile([128, AWIN], bf16, name=f"st{mb % 4}",
                                tag=f"st{mb % 4}")
                if mb in XBAR_TILES:
                    dst = st[:].rearrange("q (j p) -> q j p", p=128)
                    eng = nc.sync if mb % 2 == 0 else nc.scalar
                    eng.dma_start(dst, a2v[mb], transpose=True)
                else:
                    tp = ptp.tile([128, AWIN], bf16, tag="tp")
                    for j in range(4):
                        nc.tensor.transpose(
                            tp[:, j * 128:(j + 1) * 128],
                            a2v[mb][:, j * 128:(j + 1) * 128],
                            ident,
                        )
                    nc.vector.tensor_copy(st[:, 0:256], tp[:, 0:256])
                    nc.scalar.copy(st[:, 256:512], tp[:, 256:512])
                g = mb // 4
                first = g not in po
                if first:
                    po[g] = pout.tile([C, 512], f32, name=f"po{g}", tag="po")
                col = 128 * (mb % 4)
                for j in range(4):
                    nc.tensor.matmul(
                        po[g][:, col:col + 128],
                        xtp_tile(mb + j),
                        st[:, 128 * j:128 * (j + 1)],
                        start=(first and j == 0),
                        stop=(mb % 4 == 3 and j == 3),
                        skip_group_check=True,
                    )
                if mb % 4 == 3:
                    glo = 512 * g
                    nc.vector.tensor_copy(
                        outsb[:, glo:glo + 256], po[g][:, 0:256])
                    nc.scalar.copy(
                        outsb[:, glo + 256:glo + 512], po[g][:, 256:512])
                    eng = nc.sync if g % 2 == 0 else nc.scalar
                    eng.dma_start(out_d[:, glo:glo + 512],
                                  outsb[:, glo:glo + 512])

            cast_done = 0
            batch_it = iter(SCATTER_BATCHES)
            pending = next(batch_it)
            off = 0
            for ci, n in enumerate(XCHUNKS):
                fsl = slice(off, off + n)
                off += n
                f1 = pgen.tile([32, 512], f32, tag="f1")
                xsrc = xcm0 if ci == 0 else xcm[ci][:]
                nc.tensor.matmul(f1[:, 0:n], w1sT, xsrc,
                                 start=True, stop=True)
                if ci < 3:
                    warm(4, dep=xsrc)
                nc.scalar.activation(
                    fb[0:32, fsl], f1[:, 0:n],
                    mybir.ActivationFunctionType.Relu, bias=b1f,
                )
                for t in CHUNK_TILES[ci]:
                    kt = pkt.tile([128, 512], f32, tag="kt")
                    nc.tensor.matmul(
                        kt[:, 0:49], fb[:, 128 * t:128 * (t + 1)], w2b,
                        start=True, stop=True,
                    )
                    nc.vector.tensor_copy(kermT[:, t * NO:t * NO + 49],
                                          kt[:, 0:49])
                cast_done = CHUNK_TILES[ci][-1] + 1
                while pending is not None and pending[-1] < cast_done:
                    mbs_b = pending
                    k0 = mbs_b[0]
                    nb = len(mbs_b)
                    ab = cpool.tile([128, nb * AWIN], bf16, name=f"a2b{k0}",
                                    tag=f"a2b{k0}")
                    nc.gpsimd.local_scatter(
                        ab[:], kermT[:, k0 * NO:(k0 + nb) * NO],
                        idxt4[:, 0:nb * NO],
                        channels=128, num_elems=nb * AWIN, num_idxs=nb * NO,
                    )
                    for j, mb in enumerate(mbs_b):
                        a2v[mb] = ab[:, j * AWIN:(j + 1) * AWIN]
                    for mb in mbs_b:
                        emit_strip_and_mms(mb)
                        warm(3, dep=a2v[mb])
                    pending = next(batch_it, None)

    nc.compile()
    return nc


def _get_nc():
    if "nc" not in _STATE:
        _STATE["nc"] = _build()
    return _STATE["nc"]


def _make_idx_table():
    p_loc = np.arange(128)[:, None]
    o = np.arange(49)[None, :]
    di = o // 7 - 3
    dj = o % 7 - 3
    w_of = p_loc % 64
    idx = p_loc + 192 + 64 * di + dj
    masked = (w_of + dj < 0) | (w_of + dj >= 64)
    idx = np.where(masked, -1, idx)
    tab = np.full((128, NO), -1, dtype=np.int16)
    tab[:, :49] = idx.astype(np.int16)
    return tab


def _host_prep(x, w1, b1, bn_gamma, bn_beta, bn_mean, bn_var, w2, b2):
    x = np.asarray(x, dtype=np.float32)
    scale = np.asarray(bn_gamma) / np.sqrt(np.asarray(bn_var) + EPS)
    w1s = (np.asarray(w1) * scale[:, None]).astype(np.float32)
    b1f = (np.asarray(b1) * scale + np.asarray(bn_beta)
           - np.asarray(bn_mean) * scale).astype(np.float32)
    w1sT = np.ascontiguousarray(w1s.T).astype(BF16)            # [128, 32]
    w2b = np.vstack([np.asarray(w2, np.float32).T,
                     np.asarray(b2, np.float32)[None, :]]).astype(BF16)  # [33, 49]
    idxt = _make_idx_table()                                   # [128, 50] i16
    ident = np.eye(128, dtype=np.float32).astype(BF16)

    cb = np.zeros((128, NCB), np.uint8)
    cb[:, OFF_W1:OFF_W1 + 64] = np.ascontiguousarray(w1sT).view(np.uint8)
    cb[:, OFF_ID:OFF_ID + 256] = np.ascontiguousarray(ident).view(np.uint8)
    idxt4 = np.concatenate(
        [np.where(idxt >= 0, idxt + AWIN * j, -1) for j in range(4)],
        axis=1).astype(np.int16)
    cb[:, OFF_I2:OFF_I2 + 400] = idxt4.view(np.uint8)
    cb[0:32, OFF_B1:OFF_B1 + 4] = np.ascontiguousarray(
        b1f[:, None]).view(np.uint8)
    cb[0:33, OFF_W2:OFF_W2 + 98] = np.ascontiguousarray(w2b).view(np.uint8)

    in_maps = []
    for core in range(8):
        b, half = divmod(core, 2)
        h0 = HH * half
        xcm = np.ascontiguousarray(
            x[b, :, h0:h0 + HH, :].reshape(C, P)).astype(BF16)
        cbc = cb.copy()
        cbc[:, OFF_X0:OFF_X0 + 256] = np.ascontiguousarray(
            xcm[:, 0:128]).view(np.uint8)
        # q-space: rows h0-3 .. h0+35 (zeros outside the image)
        xe = np.zeros((C, HH + 6, W), dtype=np.float32)
        lo = max(0, h0 - 3)
        hi = min(H, h0 + HH + 3)
        xe[:, lo - (h0 - 3):hi - (h0 - 3), :] = x[b, :, lo:hi, :]
        xq = xe.reshape(C, NQT * 128).T                        # [2432, 128]
        xtp = np.ascontiguousarray(
            xq.reshape(NQT, 128, 128).transpose(1, 0, 2).reshape(128, NQT * 128)
        ).astype(BF16)
        bounds = np.cumsum((0,) + tuple(128 * len(ts) for ts in CHUNK_TILES))
        im = {f"xcm{i}": xcm[:, bounds[i]:bounds[i + 1]]
              for i in range(1, len(CHUNK_TILES))}
        im.update({"xtp0": xtp[:, :10 * 128], "xtp1": xtp[:, 10 * 128:],
                   "cb": cbc})
        in_maps.append(im)
    return in_maps


def run(inputs: dict, trace: bool = False):
    from concourse.bass_utils import run_bass_kernel_spmd

    nc = _get_nc()
    in_maps = _host_prep(**inputs)
    res = run_bass_kernel_spmd(
        nc, in_maps, core_ids=list(range(8)), trace=trace,
    )
    out = np.zeros((B, C, H, W), dtype=np.float32)
    for core in range(8):
        b, half = divmod(core, 2)
        h0 = HH * half
        out[b, :, h0:h0 + HH, :] = (
            res.results[core]["out"].astype(np.float32).reshape(C, HH, W)
        )
    return out, res


def kernel(**inputs) -> np.ndarray:
    out, _ = run(inputs, trace=False)
    return out
